# revision 1
# baseline (speedup 1.0000x reference)
"""Sparse (half-causal) multi-head attention on 8 Trainium2 NeuronCores.

Problem: x[2,2048,1024] -> QKV proj (16 heads, dk=dv=64) -> scores with
half-causal mask (rows <1024 attend cols <1024 dense; rows >=1024 causal)
-> softmax -> out proj.

Sharding: 8 cores = 2 batches x 4 head-groups (4 heads each).  Each core
computes its batch's full QKV for its 4 heads (column-sharded W), attention
for those heads, and a partial output projection (row-sharded Wo).  Host
sums the 4 partials per batch.

Per-core kernel design (all matmuls fp32r: full-rate, ~13-bit mantissa):
 - host feeds x^T so d_model lands on partitions for the projections
 - Q^T,K^T [256,2048] head-dim-on-partitions; V in natural [2048,64+1]
   layout with a ones column appended (denominator trick)
 - scores computed transposed, S^T[k,q] = K Q^T, so softmax sum over k is a
   matmul contraction: [V|1]^T P^T gives O^T stacked with the denominator
 - exp without max-subtraction (scores are O(1) by construction), 1/8 scale
   folded into the ACT activation scale
 - causal staircase handled by 4 precomputed [128,512] 0/1 mask tiles
 - biases folded in as K=1 rank-1 accumulating matmuls
"""

import sys

if "/opt/trn_rl_repo" not in sys.path:
    sys.path.insert(0, "/opt/trn_rl_repo")

import numpy as np

import concourse.bass as bass  # noqa: F401 (import registers engines)
import concourse.mybir as mybir
import concourse.tile as tile
from concourse import bacc
from concourse.bass_utils import run_bass_kernel_spmd

f32 = mybir.dt.float32
f32r = mybir.dt.float32r
AF = mybir.ActivationFunctionType
OP = mybir.AluOpType

D = 1024  # d_model
N = 2048  # n_ctx
HG = 256  # head-group width per core (4 heads x 64)


def make_tri() -> np.ndarray:
    """tri[kk, t, q'] = 1.0 if 128*t + kk <= q' else 0 — staircase masks."""
    kk = np.arange(128)[:, None, None]
    t = np.arange(4)[None, :, None]
    qp = np.arange(512)[None, None, :]
    return (128 * t + kk <= qp).astype(np.float32)


def build_nc():
    nc = bacc.Bacc("TRN2", target_bir_lowering=False, debug=False)

    xt = nc.declare_dram_parameter("xt", [D, N], f32r, isOutput=False)
    wq = nc.declare_dram_parameter("wq", [D, HG], f32r, isOutput=False)
    wk = nc.declare_dram_parameter("wk", [D, HG], f32r, isOutput=False)
    wv = nc.declare_dram_parameter("wv", [D, HG], f32r, isOutput=False)
    bqd = nc.declare_dram_parameter("bq", [HG], f32r, isOutput=False)
    bkd = nc.declare_dram_parameter("bk", [HG], f32r, isOutput=False)
    bvd = nc.declare_dram_parameter("bv", [HG], f32r, isOutput=False)
    wo = nc.declare_dram_parameter("wo", [HG, D], f32r, isOutput=False)
    trid = nc.declare_dram_parameter("tri", [128, 4, 512], f32, isOutput=False)
    onesd = nc.declare_dram_parameter("ones", [512], f32r, isOutput=False)
    y = nc.declare_dram_parameter("y", [N, D], f32, isOutput=True)
    y2 = nc.declare_dram_parameter("y2", [1024, D], f32, isOutput=True)

    dscr = nc.dram_tensor("dscr", [2, 2, 2, 1024], f32)  # (pair, parity, half)

    xt_r = xt[:].rearrange("(c p) n -> p c n", p=128)

    with tile.TileContext(nc) as tc:
        with (
            tc.tile_pool(name="persist", bufs=1) as P1,
            tc.tile_pool(name="xtp", bufs=2) as XTP,
            tc.tile_pool(name="ppool", bufs=5) as PP,
            tc.tile_pool(name="rp", bufs=2) as RP,
            tc.tile_pool(name="rbp", bufs=2) as RBP,
            tc.tile_pool(name="atp", bufs=3) as ATP,
            tc.tile_pool(name="yp", bufs=4) as YP,
            tc.tile_pool(name="ps_a", bufs=2, space="PSUM") as PSA,
            tc.tile_pool(name="ps_pv", bufs=2, space="PSUM") as PSPV,
            tc.tile_pool(name="ps_b", bufs=2, space="PSUM") as PSB,
        ):
            # ---------- constants / weights ----------
            wq_r = wq[:].rearrange("(c p) m -> p c m", p=128)
            wqa = P1.tile([128, 4, HG], f32r, tag="wqa")
            nc.sync.dma_start(wqa[:], wq_r[:, 0:4, :])
            wqb = P1.tile([128, 4, HG], f32r, tag="wqb")
            nc.sync.dma_start(wqb[:], wq_r[:, 4:8, :])
            wk_sb = P1.tile([128, 8, HG], f32r, tag="wk")
            wv_sb = P1.tile([128, 8, HG], f32r, tag="wv")
            wo_sb = P1.tile([128, 2, D], f32r, tag="wo")
            bq_sb = P1.tile([128, 2], f32, tag="bq")
            nc.gpsimd.dma_start(bq_sb[:], bqd[:].rearrange("(m p) -> p m", p=128))
            bk_sb = P1.tile([128, 2], f32, tag="bk")
            nc.gpsimd.dma_start(bk_sb[:], bkd[:].rearrange("(m p) -> p m", p=128))
            bv_sb = P1.tile([1, HG], f32r, tag="bv")
            nc.sync.dma_start(bv_sb[:], bvd[None, :])
            ones_sb = P1.tile([1, 512], f32r, tag="ones")
            nc.sync.dma_start(ones_sb[:], onesd[None, :])
            tri_sb = P1.tile([128, 4, 512], f32, tag="tri")

            qT = P1.tile([128, 2, N], f32r, tag="qT")
            kT = P1.tile([128, 2, N], f32r, tag="kT")
            v1 = P1.tile([128, 16, 4, 65], f32r, tag="v1")
            att = P1.tile([128, 2, N], f32r, tag="att")

            # ones column of [V|1] for the softmax denominator
            nc.sync.dma_start(v1[:, :, :, 64:65], onesd[0:64].partition_broadcast(128))

            # ---------- emitters ----------
            def emit_qkv_load(n4):
                ns = slice(512 * n4, 512 * n4 + 512)
                xt_n = (
                    XTP.tile([128, 4, 512], f32r, tag="xta", name=f"xta{n4}"),
                    XTP.tile([128, 4, 512], f32r, tag="xtb", name=f"xtb{n4}"),
                )
                nc.sync.dma_start(xt_n[0][:], xt_r[:, 0:4, ns])
                nc.sync.dma_start(xt_n[1][:], xt_r[:, 4:8, ns])
                return xt_n

            def emit_qkv_qk(n4, xt_n):
                """Q^T / K^T for one 512-wide seq chunk."""
                ns = slice(512 * n4, 512 * n4 + 512)
                for wget, bsb, dest in (
                    (lambda c, msl: (wqa if c < 4 else wqb)[:, c % 4, msl], bq_sb, qT),
                    (lambda c, msl: wk_sb[:, c, msl], bk_sb, kT),
                ):
                    for m in range(2):
                        msl = slice(128 * m, 128 * m + 128)
                        ps = PSB.tile([128, 512], f32, tag="b")
                        for c in range(8):
                            nc.tensor.matmul(
                                ps[:],
                                wget(c, msl),
                                xt_n[c // 4][:, c % 4, :],
                                start=(c == 0),
                                stop=(c == 7),
                            )
                        nc.vector.tensor_scalar_add(
                            dest[:, m, ns], ps[:], bsb[:, m : m + 1]
                        )

            def emit_qkv_v(n4, xt_n):
                """V natural rows for the 4 seq chunks of this n4."""
                for s in range(4 * n4, 4 * n4 + 4):
                    so = 128 * (s - 4 * n4)
                    ps = PSB.tile([128, 256], f32, tag="b")
                    for c in range(8):
                        nc.tensor.matmul(
                            ps[:],
                            xt_n[c // 4][:, c % 4, so : so + 128],
                            wv_sb[:, c, :],
                            start=(c == 0),
                            stop=False,
                        )
                    nc.tensor.matmul(
                        ps[:],
                        ones_sb[:, :128],
                        bv_sb[:],
                        start=False,
                        stop=True,
                    )
                    nc.vector.tensor_copy(
                        out=v1[:, s, :, 0:64],
                        in_=ps[:].rearrange("p (h d) -> p h d", h=4),
                    )

            def emit_qkv(n4):
                xt_n = emit_qkv_load(n4)
                emit_qkv_qk(n4, xt_n)
                emit_qkv_v(n4, xt_n)

            pv_tiles = {}

            def emit_attn_kc(hp, half, par, kc_lo, kc_hi):
                """Scores+exp+PV for one head parity over k-chunks [kc_lo, kc_hi)."""
                q0 = 1024 * half
                seg_last = [7, 7] if half == 0 else [11, 15]
                if kc_lo == 0:
                    pv_tiles[(hp, half, par)] = [
                        PSPV.tile([65, 512], f32, tag="pv", name=f"pv{hp}{half}{par}{i}")
                        for i in range(2)
                    ]
                pv = pv_tiles[(hp, half, par)]
                base = 64 * par
                for kc in range(kc_lo, kc_hi):
                    diag = half == 1 and kc >= 8
                    vq = 128 * (kc - 8) if diag else 0
                    segs = [nn for nn in range(2) if 512 * nn + 512 > vq]
                    s_t = PSA.tile(
                        [128, 1024], f32, tag="s", name=f"s{hp}{half}{par}{kc}"
                    )
                    for nn in segs:
                        qs = slice(q0 + 512 * nn, q0 + 512 * nn + 512)
                        nc.tensor.matmul(
                            s_t[:, 512 * nn : 512 * nn + 512],
                            kT[base : base + 64, hp, 128 * kc : 128 * kc + 128],
                            qT[base : base + 64, hp, qs],
                            start=True,
                            stop=True,
                        )
                    p_t = PP.tile([128, 1024], f32r, tag="p")
                    if not diag:
                        nc.scalar.activation(p_t[:], s_t[:], AF.Exp, scale=0.125)
                    else:
                        # one exp over the valid segs, then mask the diagonal
                        # seg in place (reading the f32r tile as f32 bits)
                        mseg = vq // 512
                        t = (vq - 512 * mseg) // 128
                        lo = 512 * segs[0]
                        nc.scalar.activation(
                            p_t[:, lo:1024], s_t[:, lo:1024], AF.Exp, scale=0.125
                        )
                        msl = slice(512 * mseg, 512 * mseg + 512)
                        nc.vector.tensor_tensor(
                            p_t[:, msl],
                            p_t[:, msl].bitcast(f32),
                            tri_sb[:, t, :],
                            OP.mult,
                        )
                    # PV accumulation (+ denominator row 64)
                    for nn in segs:
                        sl_ = slice(512 * nn, 512 * nn + 512)
                        nc.tensor.matmul(
                            pv[nn][0:65, :],
                            v1[:, kc, 2 * hp + par, :],
                            p_t[:, sl_],
                            start=(kc == 0),
                            stop=(kc == seg_last[nn]),
                        )

            def emit_attn_norm(hp, half, par, seg, fast=False):
                """Normalize one 512-wide q seg: att = O^T * (1/denom).  Stage
                through SBUF so the pv bank frees without waiting the denom
                broadcast.  fast=True broadcasts via a K=1 matmul into PSUM
                (no DRAM roundtrip) — used where the PE is otherwise idle."""
                q0 = 1024 * half + 512 * seg
                pv = pv_tiles[(hp, half, par)]
                sl = slice(512 * seg, 512 * seg + 512)
                stage = ATP.tile([65, 512], f32, tag="at", name=f"st{hp}{half}{par}{seg}")
                nc.vector.tensor_copy(out=stage[:], in_=pv[seg][:, :])
                if fast:
                    rr = RP.tile([1, 512], f32r, tag="r", name=f"rf{hp}{half}{par}{seg}")
                    with nc.allow_low_precision(reason="f32r denom for K=1 broadcast"):
                        nc.vector.reciprocal(rr[:], stage[64:65, :])
                    rb = PSB.tile([64, 512], f32, tag="b", name=f"rbp{hp}{half}{par}{seg}")
                    nc.tensor.matmul(rb[:], ones_sb[:, :64], rr[:], start=True, stop=True)
                else:
                    r_sb = RP.tile([1, 512], f32, tag="r", name=f"r{hp}{half}{par}{seg}")
                    nc.vector.reciprocal(r_sb[:], stage[64:65, :])
                    nc.sync.dma_start(dscr[hp, par, half, sl], r_sb[:])
                    rb = RBP.tile([64, 512], f32, tag="rb", name=f"rb{hp}{half}{par}{seg}")
                    nc.sync.dma_start(
                        rb[:], dscr[hp, par, half, sl].partition_broadcast(64)
                    )
                base = 64 * par  # DVE partition-offset write for par 1
                nc.vector.tensor_tensor(
                    att[base : base + 64, hp, q0 : q0 + 512], stage[0:64, :], rb[:], OP.mult
                )

            def emit_outproj(s_lo, s_hi, act_copies=False):
                """Partial output projection for seq chunks [s_lo, s_hi)."""
                for s in range(s_lo, s_hi):
                    yt = YP.tile([128, D], f32, tag="y", name=f"yt{s}")
                    for nseg in range(2):
                        ps = PSB.tile([128, 512], f32, tag="b", name=f"yps{s}{nseg}")
                        for hp in range(2):
                            nc.tensor.matmul(
                                ps[:],
                                att[:, hp, 128 * s : 128 * s + 128],
                                wo_sb[:, hp, 512 * nseg : 512 * nseg + 512],
                                start=(hp == 0),
                                stop=(hp == 1),
                            )
                        dst = yt[:, 512 * nseg : 512 * nseg + 512]
                        if act_copies:
                            nc.scalar.copy(out=dst, in_=ps[:])
                        else:
                            nc.vector.tensor_copy(out=dst, in_=ps[:])
                        nc.sync.dma_start(
                            y[128 * s : 128 * s + 128, 512 * nseg : 512 * nseg + 512],
                            dst,
                        )

            def emit_outproj_hp(s_lo, s_hi, hp, act_copies=False):
                """Single-head-pair out-proj pass; hp=1 accumulates into y."""
                for s in range(s_lo, s_hi):
                    yt = YP.tile([128, D], f32, tag="y", name=f"yth{s}{hp}")
                    for nseg in range(2):
                        ps = PSB.tile([128, 512], f32, tag="b", name=f"ypsh{s}{nseg}{hp}")
                        nc.tensor.matmul(
                            ps[:],
                            att[:, hp, 128 * s : 128 * s + 128],
                            wo_sb[:, hp, 512 * nseg : 512 * nseg + 512],
                            start=True,
                            stop=True,
                        )
                        dst = yt[:, 512 * nseg : 512 * nseg + 512]
                        if act_copies:
                            nc.scalar.copy(out=dst, in_=ps[:])
                        else:
                            nc.vector.tensor_copy(out=dst, in_=ps[:])
                        tgt = (
                            y[128 * s : 128 * s + 128, 512 * nseg : 512 * nseg + 512]
                            if hp == 0
                            else y2[
                                128 * (s - 8) : 128 * (s - 8) + 128,
                                512 * nseg : 512 * nseg + 512,
                            ]
                        )
                        nc.sync.dma_start(tgt, dst)

            # ---------- emission order: overlap QKV/outproj PE work with exp-bound attention ----------
            xt0 = emit_qkv_load(0)
            nc.sync.dma_start(wk_sb[:], wk[:].rearrange("(c p) m -> p c m", p=128))
            nc.sync.dma_start(wv_sb[:], wv[:].rearrange("(c p) m -> p c m", p=128))
            emit_qkv_qk(0, xt0)
            emit_qkv_v(0, xt0)
            emit_qkv(1)
            # q-half 0 of both head pairs only needs xt chunks 0-1
            for par in range(2):
                emit_attn_kc(0, 0, par, 0, 8)
                emit_attn_norm(0, 0, par, 0)
                emit_attn_norm(0, 0, par, 1)
            # non-critical loads go here: the ramp is DMA-bandwidth-bound and
            # these 6MB would delay xt1/wk/wv; DMA idles during attention
            nc.sync.dma_start(tri_sb[:], trid[:])
            nc.sync.dma_start(wo_sb[:], wo[:].rearrange("(c p) n -> p c n", p=128))
            xt2 = emit_qkv_load(2)
            xt3 = emit_qkv_load(3)
            for par in range(2):
                emit_attn_kc(1, 0, par, 0, 8)
                emit_attn_norm(1, 0, par, 0)
                emit_attn_norm(1, 0, par, 1)
            # chunk 2/3 projections gate only the half-1 attention blocks:
            # emitted after the half-0 blocks they become pure PE filler
            emit_qkv_qk(2, xt2)
            emit_qkv_qk(3, xt3)
            emit_qkv_v(2, xt2)
            emit_qkv_v(3, xt3)
            for par in range(2):
                emit_attn_kc(0, 1, par, 0, 12)
                emit_attn_norm(0, 1, par, 0)
                emit_attn_kc(0, 1, par, 12, 16)
                emit_attn_norm(0, 1, par, 1)
            # rows [0, 1024) of the output only need q-half 0 attention
            emit_outproj(0, 8)
            emit_attn_kc(1, 1, 0, 0, 12)
            emit_attn_norm(1, 1, 0, 0)
            emit_attn_kc(1, 1, 0, 12, 16)
            emit_attn_norm(1, 1, 0, 1)
            emit_attn_kc(1, 1, 1, 0, 12)
            emit_attn_norm(1, 1, 1, 0)
            emit_outproj_hp(8, 16, 0)
            emit_attn_kc(1, 1, 1, 12, 16)
            # seq chunks 8-11 only need q-seg 0 of the last block
            emit_outproj_hp(8, 12, 1, act_copies=True)
            emit_attn_norm(1, 1, 1, 1)
            emit_outproj_hp(12, 16, 1, act_copies=True)

    nc.compile()
    return nc


_NC = None
_TRI = None
_ONES = None


def _get_nc():
    global _NC, _TRI, _ONES
    if _NC is None:
        _NC = build_nc()
        _TRI = make_tri()
        _ONES = np.ones(512, np.float32)
    return _NC


def make_in_maps(x, Wq, bq, Wk, bk, Wv, bv, Wo):
    _get_nc()
    x = np.asarray(x, np.float32)
    in_maps = []
    for core in range(8):
        b, g = core // 4, core % 4
        sl = slice(HG * g, HG * (g + 1))
        in_maps.append(
            {
                "xt": np.ascontiguousarray(x[b].T),
                "wq": np.ascontiguousarray(np.asarray(Wq, np.float32)[:, sl]),
                "wk": np.ascontiguousarray(np.asarray(Wk, np.float32)[:, sl]),
                "wv": np.ascontiguousarray(np.asarray(Wv, np.float32)[:, sl]),
                "bq": np.ascontiguousarray(np.asarray(bq, np.float32)[sl]),
                "bk": np.ascontiguousarray(np.asarray(bk, np.float32)[sl]),
                "bv": np.ascontiguousarray(np.asarray(bv, np.float32)[sl]),
                "wo": np.ascontiguousarray(np.asarray(Wo, np.float32)[sl, :]),
                "tri": _TRI,
                "ones": _ONES,
            }
        )
    return in_maps


def kernel(x, Wq, bq, Wk, bk, Wv, bv, Wo, _trace=False, _trace_kwargs=None):
    nc = _get_nc()
    in_maps = make_in_maps(x, Wq, bq, Wk, bk, Wv, bv, Wo)
    res = run_bass_kernel_spmd(
        nc, in_maps, list(range(8)), trace=_trace, **(_trace_kwargs or {})
    )
    out = np.zeros((2, N, D), np.float64)
    for core in range(8):
        out[core // 4] += res.results[core]["y"].astype(np.float64)
        out[core // 4, 1024:] += res.results[core]["y2"].astype(np.float64)
    y = out.astype(np.float32)
    if _trace:
        return y, res
    return y



# revision 3
# speedup vs baseline: 1.1302x; 1.1302x over previous
"""Sparse (half-causal) multi-head attention on 8 Trainium2 NeuronCores.

Problem: x[2,2048,1024] -> QKV proj (16 heads, dk=dv=64) -> scores with
half-causal mask (rows <1024 attend cols <1024 dense; rows >=1024 causal)
-> softmax -> out proj.

Sharding: 8 cores = 2 batches x 4 head-groups (4 heads each).  Each core
computes its batch's full QKV for its 4 heads (column-sharded W), attention
for those heads, and a partial output projection (row-sharded Wo).  Host
sums the 4 partials per batch.

v2 design (bf16 everywhere off-chip and for PE moving operands):
 - q-outer attention: per 512-wide q seg, scores S^T[k,q] land in PSUM
   [128,2,512] tiles (one per kc pair), exp'd 1024-wide into bf16 p tiles
 - PV in q-major layout: out[q,dv] = P^T-slicesT @ [V|1] with F=65 (bf16,
   1 cyc/row), accumulated per 128-q-tile in PSUM [128,4,65]; column 64 is
   the softmax denominator (ones column of V)
 - normalization is per-PARTITION (q on partitions): DVE reciprocal +
   tensor_scalar_mul, no partition-broadcast roundtrips at all
 - O[q,dv] -> O^T via the DMA XBAR transpose (2 heads staged side by side
   to honor the 128-col tile constraint); out-proj then runs from O^T
 - causal staircase via bf16 tri masks on the (finite) exp'd diag chunks
 - PE-filler queue: QKV/out-proj matmuls are interleaved between attention
   pairs so the PE never waits on the (Act-bound) exp stream
"""

import sys

if "/opt/trn_rl_repo" not in sys.path:
    sys.path.insert(0, "/opt/trn_rl_repo")

import ml_dtypes
import numpy as np

import concourse.bass as bass  # noqa: F401 (import registers engines)
import concourse.mybir as mybir
import concourse.tile as tile
from concourse import bacc
from concourse.bass_utils import run_bass_kernel_spmd

f32 = mybir.dt.float32
bf16 = mybir.dt.bfloat16
AF = mybir.ActivationFunctionType
OP = mybir.AluOpType

D = 1024  # d_model
N = 2048  # n_ctx
HG = 256  # head-group width per core (4 heads x 64)

# PE rows of filler emitted per attention pair (matches the ~2.1us the Act
# engine spends on the pair's two exps, minus the pair's own PE work)
FILL_ROWS = 2000


def make_tri() -> np.ndarray:
    """tri[kk, t, q'] = 1.0 if 128*t + kk <= q' else 0 — staircase masks."""
    kk = np.arange(128)[:, None, None]
    t = np.arange(4)[None, :, None]
    qp = np.arange(512)[None, None, :]
    return (128 * t + kk <= qp).astype(ml_dtypes.bfloat16)


def build_nc():
    nc = bacc.Bacc("TRN2", target_bir_lowering=False, debug=False)

    xt = nc.declare_dram_parameter("xt", [D, N], bf16, isOutput=False)
    wq = nc.declare_dram_parameter("wq", [D, HG], bf16, isOutput=False)
    wk = nc.declare_dram_parameter("wk", [D, HG], bf16, isOutput=False)
    wv = nc.declare_dram_parameter("wv", [D, HG], bf16, isOutput=False)
    bqd = nc.declare_dram_parameter("bq", [HG], f32, isOutput=False)
    bkd = nc.declare_dram_parameter("bk", [HG], f32, isOutput=False)
    bvd = nc.declare_dram_parameter("bv", [HG], bf16, isOutput=False)
    wo = nc.declare_dram_parameter("wo", [HG, D], bf16, isOutput=False)
    trid = nc.declare_dram_parameter("tri", [128, 4, 512], bf16, isOutput=False)
    y = nc.declare_dram_parameter("y", [N, D], bf16, isOutput=True)

    xt_r = xt[:].rearrange("(c p) n -> p c n", p=128)

    with tile.TileContext(nc) as tc:
        with (
            tc.tile_pool(name="persist", bufs=1) as P1,
            tc.tile_pool(name="xtp", bufs=2) as XTP,
            tc.tile_pool(name="pp", bufs=3) as PP,
            tc.tile_pool(name="stg", bufs=2) as STG,
            tc.tile_pool(name="rp", bufs=2) as RP,
            tc.tile_pool(name="yp", bufs=2) as YP,
            tc.tile_pool(name="ps_s", bufs=2, space="PSUM") as PSA,
            tc.tile_pool(name="ps_pv", bufs=2, space="PSUM") as PVQ,
            tc.tile_pool(name="ps_b", bufs=2, space="PSUM") as PSB,
        ):
            # ---------- persistent tiles ----------
            wq_sb = P1.tile([128, 8, HG], bf16, tag="wq")
            wk_sb = P1.tile([128, 8, HG], bf16, tag="wk")
            wv_sb = P1.tile([128, 8, HG], bf16, tag="wv")
            wo_sb = P1.tile([128, 2, D], bf16, tag="wo")
            bq_sb = P1.tile([128, 2], f32, tag="bq")
            bk_sb = P1.tile([128, 2], f32, tag="bk")
            bv_sb = P1.tile([1, HG], bf16, tag="bv")
            ones_sb = P1.tile([1, 128], bf16, tag="ones")
            tri_sb = P1.tile([128, 4, 512], bf16, tag="tri")

            qT = P1.tile([128, 2, N], bf16, tag="qT")
            kT = P1.tile([128, 2, N], bf16, tag="kT")
            v1 = P1.tile([128, 16, 4, 65], bf16, tag="v1")
            att = P1.tile([128, 2, N], bf16, tag="att")

            # ---------- initial loads ----------
            nc.sync.dma_start(wq_sb[:], wq[:].rearrange("(c p) m -> p c m", p=128))

            def emit_xt_load(n4):
                ns = slice(512 * n4, 512 * n4 + 512)
                xa = XTP.tile([128, 4, 512], bf16, tag="xta", name=f"xta{n4}")
                xb = XTP.tile([128, 4, 512], bf16, tag="xtb", name=f"xtb{n4}")
                nc.sync.dma_start(xa[:], xt_r[:, 0:4, ns])
                nc.sync.dma_start(xb[:], xt_r[:, 4:8, ns])
                return (xa, xb)

            xts = {0: emit_xt_load(0)}
            nc.sync.dma_start(wk_sb[:], wk[:].rearrange("(c p) m -> p c m", p=128))
            nc.sync.dma_start(wv_sb[:], wv[:].rearrange("(c p) m -> p c m", p=128))
            nc.gpsimd.dma_start(bq_sb[:], bqd[:].rearrange("(m p) -> p m", p=128))
            nc.gpsimd.dma_start(bk_sb[:], bkd[:].rearrange("(m p) -> p m", p=128))
            nc.sync.dma_start(bv_sb[:], bvd[None, :])
            nc.gpsimd.memset(ones_sb[:], 1.0)
            for h in range(4):
                nc.gpsimd.memset(v1[:, :, h, 64:65], 1.0)

            # ---------- QKV / out-proj piece emitters ----------
            def emit_qk_piece(which, m, n4):
                xt_n = xts[n4]
                ns = slice(512 * n4, 512 * n4 + 512)
                msl = slice(128 * m, 128 * m + 128)
                wsb, bsb, dest = (
                    (wq_sb, bq_sb, qT) if which == "q" else (wk_sb, bk_sb, kT)
                )
                ps = PSB.tile([128, 512], f32, tag="b", name=f"{which}ps{m}{n4}")
                for c in range(8):
                    nc.tensor.matmul(
                        ps[:],
                        wsb[:, c, msl],
                        xt_n[c // 4][:, c % 4, :],
                        start=(c == 0),
                        stop=(c == 7),
                    )
                nc.vector.tensor_scalar_add(dest[:, m, ns], ps[:], bsb[:, m : m + 1])

            def emit_v_piece(sch):
                xt_n = xts[sch // 4]
                so = 128 * (sch % 4)
                ps = PSB.tile([128, 256], f32, tag="b", name=f"vps{sch}")
                for c in range(8):
                    nc.tensor.matmul(
                        ps[:],
                        xt_n[c // 4][:, c % 4, so : so + 128],
                        wv_sb[:, c, :],
                        start=(c == 0),
                        stop=False,
                    )
                nc.tensor.matmul(ps[:], ones_sb[:], bv_sb[:], start=False, stop=True)
                nc.vector.tensor_copy(
                    out=v1[:, sch, :, 0:64],
                    in_=ps[:].rearrange("p (h d) -> p h d", h=4),
                )

            def emit_outproj_tile(T):
                yt = YP.tile([128, D], bf16, tag="y", name=f"yt{T}")
                for dseg in range(2):
                    ps = PSB.tile([128, 512], f32, tag="b", name=f"yps{T}{dseg}")
                    for hp in range(2):
                        nc.tensor.matmul(
                            ps[:],
                            att[:, hp, 128 * T : 128 * T + 128],
                            wo_sb[:, hp, 512 * dseg : 512 * dseg + 512],
                            start=(hp == 0),
                            stop=(hp == 1),
                        )
                    nc.vector.tensor_copy(out=yt[:, 512 * dseg : 512 * dseg + 512], in_=ps[:])
                nc.sync.dma_start(y[128 * T : 128 * T + 128, :], yt[:])

            # ---------- PE filler queue ----------
            pending = []  # entries [key, rows, fn], key like ('k', n4) / ('v', sch) / ('op', T)

            def push(key, rows, fn):
                pending.append((key, rows, fn))

            def require(pred):
                rest = []
                for e in pending:
                    if pred(e[0]):
                        e[2]()
                    else:
                        rest.append(e)
                pending[:] = rest

            def fill(budget):
                while pending and budget > 0:
                    key, rows, fn = pending.pop(0)
                    fn()
                    budget -= rows

            def push_qkv(n4):
                for m in range(2):
                    push(("k", n4), 4096, lambda m=m: emit_qk_piece("k", m, n4))
                for sch in range(4 * n4, 4 * n4 + 4):
                    push(("v", sch), 2560, lambda sch=sch: emit_v_piece(sch))
                for m in range(2):
                    push(("q", n4), 4096, lambda m=m: emit_qk_piece("q", m, n4))

            # ---------- attention substream ----------
            def emit_substream(half, s, hp):
                q0 = 1024 * half + 512 * s
                n4q = q0 // 512
                n_kc = 8 if half == 0 else 12 + 4 * s
                diag0 = 8 + 4 * s  # first diag kc (half 1 only)
                kc_max = [7 if half == 0 else 8 + 4 * s + t for t in range(4)]
                require(lambda k: k[0] == "q" and k[1] == n4q)
                pvq = {
                    par: PVQ.tile(
                        [128, 4, 65], f32, tag="pvq", name=f"pvq{half}{s}{hp}{par}"
                    )
                    for par in (0, 1)
                }
                for p in range(n_kc // 2):
                    require(
                        lambda k, p=p: (k[0] == "k" and k[1] <= (2 * p + 1) // 4)
                        or (k[0] == "v" and k[1] <= 2 * p + 1)
                    )
                    s_ps, pts = {}, {}
                    for par in (0, 1):
                        base = 64 * par
                        st = PSA.tile(
                            [128, 2, 512], f32, tag="s", name=f"s{half}{s}{hp}{par}{p}"
                        )
                        for j, kc in enumerate((2 * p, 2 * p + 1)):
                            nc.tensor.matmul(
                                st[:, j, :],
                                kT[base : base + 64, hp, 128 * kc : 128 * kc + 128],
                                qT[base : base + 64, hp, q0 : q0 + 512],
                                start=True,
                                stop=True,
                            )
                        s_ps[par] = st
                    for par in (0, 1):
                        pt = PP.tile(
                            [128, 2, 512], bf16, tag="p", name=f"p{half}{s}{hp}{par}{p}"
                        )
                        nc.scalar.activation(pt[:], s_ps[par][:], AF.Exp, scale=0.125)
                        if half == 1:
                            for j, kc in enumerate((2 * p, 2 * p + 1)):
                                if kc >= diag0:
                                    nc.vector.tensor_tensor(
                                        pt[:, j, :],
                                        pt[:, j, :],
                                        tri_sb[:, kc - diag0, :],
                                        OP.mult,
                                    )
                        pts[par] = pt
                    for par in (0, 1):
                        for t in range(4):
                            for j, kc in enumerate((2 * p, 2 * p + 1)):
                                if kc <= kc_max[t]:
                                    # start only on the bank's very first write:
                                    # it marks the whole 2KB bank pending-zero,
                                    # so each t-group's first write replaces
                                    # (self-zeroes) and later writes accumulate.
                                    nc.tensor.matmul(
                                        pvq[par][:, t, :],
                                        pts[par][:, j, 128 * t : 128 * t + 128],
                                        v1[:, kc, 2 * hp + par, :],
                                        start=(kc == 0 and t == 0),
                                        stop=(kc == kc_max[t]),
                                        skip_group_check=(not (kc == 0 and t == 0)),
                                    )
                    fill(FILL_ROWS)
                # normalize + transpose into att
                stage = STG.tile([128, 4, 128], bf16, tag="stg", name=f"stg{half}{s}{hp}")
                for par in (0, 1):
                    rcp = RP.tile([128, 4], f32, tag="rcp", name=f"rcp{half}{s}{hp}{par}")
                    nc.vector.reciprocal(rcp[:], pvq[par][:, :, 64])
                    for t in range(4):
                        nc.vector.tensor_scalar_mul(
                            stage[:, t, 64 * par : 64 * par + 64],
                            pvq[par][:, t, 0:64],
                            rcp[:, t : t + 1],
                        )
                for t in range(4):
                    nc.sync.dma_start(
                        att[:, hp, q0 + 128 * t : q0 + 128 * t + 128],
                        stage[:, t, :],
                        transpose=True,
                    )

            # ---------- main emission ----------
            for m in range(2):
                emit_qk_piece("q", m, 0)
            for m in range(2):
                emit_qk_piece("k", m, 0)
            for sch in range(4):
                emit_v_piece(sch)
            xts[1] = emit_xt_load(1)
            nc.sync.dma_start(tri_sb[:], trid[:])
            nc.sync.dma_start(wo_sb[:], wo[:].rearrange("(c p) n -> p c n", p=128))
            push_qkv(1)

            for half, s in ((0, 0), (0, 1), (1, 0), (1, 1)):
                if (half, s) == (0, 1):
                    xts[2] = emit_xt_load(2)
                    push_qkv(2)
                if (half, s) == (1, 0):
                    xts[3] = emit_xt_load(3)
                    push_qkv(3)
                for hp in (0, 1):
                    emit_substream(half, s, hp)
                for t in range(4):
                    T = 8 * half + 4 * s + t
                    push(("op", T), 2048, lambda T=T: emit_outproj_tile(T))

            require(lambda k: True)

    nc.compile()
    return nc


_NC = None
_TRI = None


def _get_nc():
    global _NC, _TRI
    if _NC is None:
        _NC = build_nc()
        _TRI = make_tri()
    return _NC


def make_in_maps(x, Wq, bq, Wk, bk, Wv, bv, Wo):
    _get_nc()
    bf = ml_dtypes.bfloat16
    x = np.asarray(x, np.float32)
    in_maps = []
    for core in range(8):
        b, g = core // 4, core % 4
        sl = slice(HG * g, HG * (g + 1))
        in_maps.append(
            {
                "xt": np.ascontiguousarray(x[b].T.astype(bf)),
                "wq": np.ascontiguousarray(np.asarray(Wq, np.float32)[:, sl].astype(bf)),
                "wk": np.ascontiguousarray(np.asarray(Wk, np.float32)[:, sl].astype(bf)),
                "wv": np.ascontiguousarray(np.asarray(Wv, np.float32)[:, sl].astype(bf)),
                "bq": np.ascontiguousarray(np.asarray(bq, np.float32)[sl]),
                "bk": np.ascontiguousarray(np.asarray(bk, np.float32)[sl]),
                "bv": np.ascontiguousarray(np.asarray(bv, np.float32)[sl].astype(bf)),
                "wo": np.ascontiguousarray(np.asarray(Wo, np.float32)[sl, :].astype(bf)),
                "tri": _TRI,
            }
        )
    return in_maps


def kernel(x, Wq, bq, Wk, bk, Wv, bv, Wo, _trace=False, _trace_kwargs=None):
    nc = _get_nc()
    in_maps = make_in_maps(x, Wq, bq, Wk, bk, Wv, bv, Wo)
    res = run_bass_kernel_spmd(
        nc, in_maps, list(range(8)), trace=_trace, **(_trace_kwargs or {})
    )
    out = np.zeros((2, N, D), np.float64)
    for core in range(8):
        out[core // 4] += np.asarray(res.results[core]["y"]).astype(np.float64)
    y = out.astype(np.float32)
    if _trace:
        return y, res
    return y
